# revision 23
# baseline (speedup 1.0000x reference)
"""Trainium2 Bass kernel for HNet attention (B=4, S=2048, H=768, 12 heads, RoPE, causal).

Sharding: 8 cores = 4 batches x 2 head-groups (6 heads each).
Wq/Wk/Wv split column-wise (head axis), Wo row-wise; host sums the two
partial o_proj outputs per batch (the "all-reduce" done at gather time).

Per-core dataflow (v9 — fp16, 256-wide q strips, kc-pair merged exp):
  xT [768,2048] fp16 (host-packed) --PE--> Q,K,V natural [2048,384]
  RoPE on Q,K natural (DVE muls, Pool add), PE-transpose -> qT,kT fp16
  scoresT[k,q] = kT.T @ qT per (pair m, par); causal mask folded into PE
    as an accumulate-matmul of a -30000 triangle; two kc blocks share one
    [128,1024] psum tile so one ScalarE exp covers 4 regions
  ex = exp(0.125*scores - 4.5) fp16 (bias cancels in softmax, keeps the
    self-attention diagonal e^{~14} inside fp16 range)
  PV natural per q-tile: po[q, 65] += ex.T @ [V_h | 1] (col 64 = sums),
    one psum accumulation group per head at a time
  normalize: DVE copy psum->sbuf + reciprocal, per-head scale on Pool
  deferred fill: PE-transpose -> aoT, o_proj fin = aoT.T @ woT, store.
"""

import os
import sys

import numpy as np

sys.path.insert(0, "/opt/trn_rl_repo")

from collections import deque
from contextlib import ExitStack

import concourse.bacc as bacc
import concourse.tile as tile
from concourse import mybir
from concourse.bass_utils import run_bass_kernel_spmd

S = 2048
HID = 768
NH = 6            # heads per core
D = 64
F = NH * D        # 384 per-core feature slice
P = 128
SC = S // P       # 16 s-tiles
FC = HID // P     # 6 contraction chunks
MC = F // P       # 3 head-pair chunks
QW = 256          # q strip width
NQ = S // QW      # 8 strips
QT = QW // P      # 2 q-tiles per strip
N_CORES = 8
ROPE_THETA = 10000.0
NEG = -30000.0
EBIAS = -4.5      # exp bias: cancels in softmax, keeps ex within fp16

F32 = mybir.dt.float32
F16 = mybir.dt.float16
AF = mybir.ActivationFunctionType

XW = 4 * FC * QW * 2   # packed xT width 12288 (4 col-chunks x 6 f x 512)
XCH = FC * 512         # 3072 per chunk
WW = FC * 3 * F        # packed wqkv width 6912
CW = SC * 2 * F        # packed cos|sin width 12288


def build_program():
    nc = bacc.Bacc("TRN2", target_bir_lowering=False, debug=False,
                   num_devices=N_CORES)

    xTp_d = nc.dram_tensor("xTp", [P, XW], F16, kind="ExternalInput").ap()
    wp_d = nc.dram_tensor("wp", [P, WW], F16, kind="ExternalInput").ap()
    woT_d = nc.dram_tensor("woT", [P, MC * HID], F16, kind="ExternalInput").ap()
    csn_d = nc.dram_tensor("csn", [P, CW], F16, kind="ExternalInput").ap()
    tri_d = nc.dram_tensor("trineg", [P, P], F16, kind="ExternalInput").ap()
    eye_d = nc.dram_tensor("eye", [P, P], F16, kind="ExternalInput").ap()
    out_d = nc.dram_tensor("out", [S, HID], F32, kind="ExternalOutput").ap()

    with tile.TileContext(nc) as tc, ExitStack() as ctx:
        const_pool = ctx.enter_context(tc.tile_pool(name="const", bufs=1))
        eye_sb = const_pool.tile([P, P], F16, tag="eye")
        nc.sync.dma_start(eye_sb[:], eye_d[:])
        tri_sb = const_pool.tile([P, P], F16, tag="tri")
        nc.sync.dma_start(tri_sb[:], tri_d[:])
        eb_sb = const_pool.tile([P, 1], F32, tag="ebias")
        nc.gpsimd.memset(eb_sb[:], EBIAS)

        # ---- persistent SBUF; DMA order feeds the prologue first ----
        xw_pool = ctx.enter_context(tc.tile_pool(name="xw", bufs=1))
        wp = xw_pool.tile([P, WW], F16, tag="wp")
        xTp = xw_pool.tile([P, XW], F16, tag="xTp")
        csn = xw_pool.tile([P, CW], F16, tag="csn")
        woT = xw_pool.tile([P, MC * HID], F16, tag="woT")
        for f in range(FC):
            c0 = f * 3 * F
            nc.sync.dma_start(wp[:, c0:c0 + 3 * F], wp_d[:, c0:c0 + 3 * F])
        nc.sync.dma_start(xTp[:, 0:XCH], xTp_d[:, 0:XCH])
        nc.sync.dma_start(csn[:, 0:CW // 4], csn_d[:, 0:CW // 4])
        nc.sync.dma_start(woT[:], woT_d[:])

        def wslice(kind, f):  # 0=q 1=k 2=v
            c0 = f * 3 * F + kind * F
            return wp[:, c0:c0 + F]

        def xslice(f, s):
            c0 = (s // 4) * XCH + f * 512 + (s % 4) * P
            return xTp[:, c0:c0 + P]

        kT_pool = ctx.enter_context(tc.tile_pool(name="kTp", bufs=1))
        kTs = kT_pool.tile([P, MC * S], F16, tag="kTs")
        v_pool = ctx.enter_context(tc.tile_pool(name="vp", bufs=1))
        vo = [v_pool.tile([P, NH * 65], F16, tag=f"v{s}", name=f"v{s}")
              for s in range(SC)]
        for s in range(SC):
            v3 = vo[s].rearrange("p (h e) -> p h e", h=NH)
            nc.gpsimd.memset(v3[:, :, 64], 1.0)

        with tc.tile_pool(name="rp", bufs=3) as rp_pool, \
             tc.tile_pool(name="qr", bufs=6) as qr_pool, \
             tc.tile_pool(name="qTs", bufs=4) as qTs_pool, \
             tc.tile_pool(name="ao", bufs=4) as ao_pool, \
             tc.tile_pool(name="ex", bufs=34) as ex_pool, \
             tc.tile_pool(name="an", bufs=6) as an_pool, \
             tc.tile_pool(name="iv", bufs=4) as iv_pool, \
             tc.tile_pool(name="ob", bufs=4) as ob_pool, \
             tc.tile_pool(name="mx", bufs=2, space="PSUM") as mx, \
             tc.tile_pool(name="sc", bufs=2, space="PSUM") as scp, \
             tc.tile_pool(name="po", bufs=2, space="PSUM") as pop:

            qTs = {}   # strip qc -> [P, MC*QW] tile
            aoT = {}   # strip qc -> [P, MC*QW] tile
            ans = {}   # (qc, qt) -> normalized ao_nat tile
            sps = {}   # (pair_index, m) -> scores psum pair tile
            exs = {}   # (pair_index, m) -> ex tile [P, 1024]
            pvb = {}   # live projB state per s

            def rope(pp, s):
                """psum QKV chunk [P, F] -> rotated fp16 sbuf tile."""
                cs = csn[:, s * 2 * F:s * 2 * F + F]
                sn = csn[:, s * 2 * F + F:s * 2 * F + 2 * F]
                p3 = pp.rearrange("p (h d) -> p h d", h=NH)
                s3 = sn.rearrange("p (h d) -> p h d", h=NH)
                t1 = rp_pool.tile([P, F], F32, tag="t1", name="t1")
                nc.vector.tensor_mul(t1[:], pp[:], cs[:])
                t2 = rp_pool.tile([P, F], F32, tag="t2", name="t2")
                t23 = t2.rearrange("p (h d) -> p h d", h=NH)
                nc.vector.tensor_mul(t23[:, :, 0:32], p3[:, :, 32:64],
                                     s3[:, :, 0:32])
                nc.vector.tensor_mul(t23[:, :, 32:64], p3[:, :, 0:32],
                                     s3[:, :, 32:64])
                qr = qr_pool.tile([P, F], F16, tag="qr", name="qr")
                nc.gpsimd.tensor_add(qr[:], t1[:], t2[:])
                return qr

            def emit_projA(s):
                qc = s // QT
                if s % QT == 0:
                    qTs[qc] = qTs_pool.tile([P, MC * QW], F16, tag="qTs",
                                            name="qTs")
                pq = mx.tile([P, F], F32, tag="mx", name="pq")
                for f in range(FC):
                    nc.tensor.matmul(pq[:], xslice(f, s), wslice(0, f),
                                     start=(f == 0), stop=(f == FC - 1))
                pk = mx.tile([P, F], F32, tag="mx", name="pk")
                for f in range(FC):
                    nc.tensor.matmul(pk[:], xslice(f, s), wslice(1, f),
                                     start=(f == 0), stop=(f == FC - 1))
                qr = rope(pq, s)
                return (s, pk, qr)

            def emit_projB(state):
                s, pk, qr = state
                qc, scol = s // QT, (s % QT) * P
                pv_ = mx.tile([P, F], F32, tag="mx", name="pv")
                for f in range(FC):
                    nc.tensor.matmul(pv_[:], xslice(f, s), wslice(2, f),
                                     start=(f == 0), stop=(f == FC - 1))
                kr = rope(pk, s)
                tq = mx.tile([P, F], F16, tag="mx", name="tpq")
                for m in range(MC):
                    nc.tensor.transpose(tq[:, m * P:(m + 1) * P],
                                        qr[:, m * P:(m + 1) * P], eye_sb[:])
                qd = qTs[qc].rearrange("p (m c) -> p m c", m=MC)
                nc.vector.tensor_copy(qd[:, :, scol:scol + P],
                                      tq.rearrange("p (m c) -> p m c", m=MC))
                tk = mx.tile([P, F], F16, tag="mx", name="tpk")
                for m in range(MC):
                    nc.tensor.transpose(tk[:, m * P:(m + 1) * P],
                                        kr[:, m * P:(m + 1) * P], eye_sb[:])
                kd = kTs.rearrange("p (m c) -> p m c", m=MC)
                nc.vector.tensor_copy(kd[:, :, s * P:(s + 1) * P],
                                      tk.rearrange("p (m c) -> p m c", m=MC))
                v3 = vo[s].rearrange("p (h e) -> p h e", h=NH)
                p3 = pv_.rearrange("p (h d) -> p h d", h=NH)
                nc.vector.tensor_copy(v3[:, :, 0:64], p3[:])

            def emit_scores(qc, kc, m):
                """scores for block kc into the kc-pair psum tile; emit the
                exp(s) when the pair completes (odd kc)."""
                q0, k0 = qc * QW, kc * P
                off = max(0, k0 - q0)
                kcp = kc // 2
                odd = kc & 1
                if not odd:
                    sps[(kcp, m)] = scp.tile([P, 4 * QW], F32, tag="sc",
                                             name="sp")
                sp = sps[(kcp, m)]
                b0 = odd * 2 * QW
                for par in range(2):
                    b = b0 + par * QW
                    d0 = 64 * par
                    lhsT = kTs[d0:d0 + 64, m * S + k0:m * S + k0 + P]
                    rhs = qTs[qc]
                    r0 = m * QW
                    if k0 >= q0:  # diagonal block: fold mask into PE
                        nc.tensor.matmul(sp[:, b + off:b + off + P],
                                         lhsT,
                                         rhs[d0:d0 + 64, r0 + off:r0 + off + P],
                                         start=True, stop=False)
                        nc.tensor.matmul(sp[:, b + off:b + off + P],
                                         eye_sb[:], tri_sb[:],
                                         start=False, stop=True)
                        if off + P < QW:
                            nc.tensor.matmul(sp[:, b + off + P:b + QW],
                                             lhsT,
                                             rhs[d0:d0 + 64, r0 + off + P:
                                                 r0 + QW],
                                             start=True, stop=True)
                    else:
                        nc.tensor.matmul(sp[:, b:b + QW],
                                         lhsT, rhs[d0:d0 + 64, r0:r0 + QW],
                                         start=True, stop=True)
                if odd:
                    ex = ex_pool.tile([P, 4 * QW], F16, tag="ex", name="ex")
                    last_pair = (kc == 2 * qc + 1)
                    if last_pair:
                        # ragged diag pair: separate exp per kc block
                        for o, ofe in ((0, 0), (1, P)):
                            sp3 = sp.rearrange("p (r c) -> p r c", r=4)
                            ex3 = ex.rearrange("p (r c) -> p r c", r=4)
                            nc.scalar.activation(
                                ex3[:, 2 * o:2 * o + 2, ofe:QW],
                                sp3[:, 2 * o:2 * o + 2, ofe:QW],
                                AF.Exp, scale=0.125, bias=eb_sb[:])
                    else:
                        sp3 = sp.rearrange("p (r c) -> p r c", r=4)
                        ex3 = ex.rearrange("p (r c) -> p r c", r=4)
                        nc.scalar.activation(ex3[:], sp3[:],
                                             AF.Exp, scale=0.125, bias=eb_sb[:])
                    exs[(kcp, m)] = ex
                    del sps[(kcp, m)]

            def emit_pv(qc, qt):
                """one psum accumulation group per head at a time."""
                t = QT * qc + qt
                po = pop.tile([P, NH * 65], F32, tag="po", name="po")
                for m in range(MC):
                    for par in range(2):
                        h = 2 * m + par
                        for kc in range(t + 1):
                            ex = exs[(kc // 2, m)]
                            c = (kc & 1) * 2 * QW + par * QW + qt * P
                            nc.tensor.matmul(po[:, h * 65:h * 65 + 65],
                                             ex[:, c:c + P],
                                             vo[kc][:, h * 65:h * 65 + 65],
                                             start=(kc == 0), stop=(kc == t))
                return po

            def emit_norm(qc, qt, po):
                """copy po to SBUF, reciprocal, per-head scale on Pool."""
                pz = iv_pool.tile([P, NH * 65], F32, tag="pz", name="pz")
                nc.vector.tensor_copy(pz[:], po[:])
                pz3 = pz.rearrange("p (h e) -> p h e", h=NH)
                inv = iv_pool.tile([P, NH], F32, tag="inv", name="inv")
                with nc.allow_low_precision(reason="softmax sums"):
                    nc.vector.reciprocal(inv[:], pz3[:, :, 64])
                an = an_pool.tile([P, F], F16, tag="an", name="an")
                for h in range(NH):
                    nc.gpsimd.tensor_scalar_mul(an[:, h * D:(h + 1) * D],
                                                pz3[:, h, 0:D],
                                                inv[:, h:h + 1])
                ans[(qc, qt)] = an

            def emit_oproj(qc, qt):
                """deferred PE fill: transpose ao_nat -> aoT, fin, store."""
                if qt == 0:
                    aoT[qc] = ao_pool.tile([P, MC * QW], F16, tag="aoT",
                                           name="aoT")
                an = ans.pop((qc, qt))
                ta = mx.tile([P, F], F16, tag="mx", name="tpa")
                for m in range(MC):
                    nc.tensor.transpose(ta[:, m * P:(m + 1) * P],
                                        an[:, m * P:(m + 1) * P], eye_sb[:])
                ad = aoT[qc].rearrange("p (m c) -> p m c", m=MC)
                nc.vector.tensor_copy(ad[:, :, qt * P:(qt + 1) * P],
                                      ta.rearrange("p (m c) -> p m c", m=MC))
                ob = ob_pool.tile([P, HID], F32, tag="ob", name="ob")
                for half in range(2):
                    c0 = half * F
                    fin = scp.tile([P, F], F32, tag="sc", name="fin")
                    for m in range(MC):
                        nc.tensor.matmul(fin[:],
                                         aoT[qc][:, m * QW + qt * P:
                                                 m * QW + (qt + 1) * P],
                                         woT[:, m * HID + c0:m * HID + c0 + F],
                                         start=(m == 0), stop=(m == MC - 1))
                    nc.vector.tensor_copy(ob[:, c0:c0 + F], fin[:])
                s0 = (QT * qc + qt) * P
                nc.sync.dma_start(out_d[s0:s0 + P, :], ob[:])

            # ---- emission schedule ----
            fills = deque()
            for s in range(2, SC):
                fills.append(("A", s))
                fills.append(("B", s))

            def pop_fill():
                if not fills:
                    return
                kind, a = fills.popleft()
                if kind == "A":
                    pvb[a] = emit_projA(a)
                elif kind == "B":
                    emit_projB(pvb.pop(a))
                else:
                    emit_oproj(*a)

            for s in range(2):
                st = emit_projA(s)
                emit_projB(st)
            for c in range(1, 4):
                nc.sync.dma_start(xTp[:, c * XCH:(c + 1) * XCH],
                                  xTp_d[:, c * XCH:(c + 1) * XCH])
                nc.sync.dma_start(csn[:, c * (CW // 4):(c + 1) * (CW // 4)],
                                  csn_d[:, c * (CW // 4):(c + 1) * (CW // 4)])

            for qc in range(NQ):
                exs.clear()
                last = QT * qc + 1
                # strip qc's scores read qTs[qc] whole: its proj must be done
                while any(k in ("A", "B") and a <= last for k, a in fills):
                    pop_fill()
                for kcp in range(qc + 1):
                    pop_fill()
                    pop_fill()
                    for m in range(MC):
                        emit_scores(qc, 2 * kcp, m)
                        emit_scores(qc, 2 * kcp + 1, m)
                for qt in range(QT):
                    po = emit_pv(qc, qt)
                    emit_norm(qc, qt, po)
                    fills.append(("O", (qc, qt)))
            while fills:
                pop_fill()
    nc.compile()
    return nc


def _rope_tables():
    inv_freq = 1.0 / (ROPE_THETA ** (np.arange(0, D, 2, dtype=np.float32) / D))
    t = np.arange(S, dtype=np.float32)
    freqs = np.outer(t, inv_freq)                       # [S, 32]
    emb = np.concatenate([freqs, freqs], axis=-1)       # [S, 64]
    cos = np.cos(emb).astype(np.float32)
    sin = np.sin(emb).astype(np.float32)
    sin_signed = sin.copy()
    sin_signed[:, 0:32] *= -1.0                         # fold rotate_half sign
    cos6 = np.tile(cos, (1, NH))                        # [S, 384]
    sin6 = np.tile(sin_signed, (1, NH))
    both = np.concatenate(
        [cos6.reshape(SC, P, F), sin6.reshape(SC, P, F)], axis=2)
    return np.ascontiguousarray(
        both.transpose(1, 0, 2).reshape(P, CW)).astype(np.float16)


_STATE = {}


def _get_program():
    if "nc" not in _STATE:
        _STATE["nc"] = build_program()
    return _STATE["nc"]


def _pack_x(xT):
    """[768, 2048] -> [128, 12288] with cols (chunk, f, 512)."""
    v = xT.reshape(FC, P, 4, 512)                # f, p, c, col
    return np.ascontiguousarray(
        v.transpose(1, 2, 0, 3).reshape(P, XW))  # p, (c f col)


def _pack_w(Wq, Wk, Wv, cols):
    ws = [np.asarray(W[cols, :].T, dtype=np.float32).reshape(FC, P, F)
          for W in (Wq, Wk, Wv)]
    stk = np.stack(ws, axis=2)                   # f, p, kind, 384
    return np.ascontiguousarray(stk.transpose(1, 0, 2, 3).reshape(P, WW))


def _make_in_maps(hidden_states, Wq, Wk, Wv, Wo):
    hs = np.asarray(hidden_states, dtype=np.float32)
    Wq = np.asarray(Wq, dtype=np.float32)
    Wk = np.asarray(Wk, dtype=np.float32)
    Wv = np.asarray(Wv, dtype=np.float32)
    Wo = np.asarray(Wo, dtype=np.float32)

    csn = _rope_tables()
    trineg = (NEG * np.tril(np.ones((P, P), dtype=np.float32), -1)
              ).astype(np.float16)
    eye = np.eye(P, dtype=np.float16)

    in_maps = []
    for c in range(N_CORES):
        b, g = c // 2, c % 2
        cols = slice(g * F, (g + 1) * F)
        woT = np.asarray(Wo[:, cols].T, dtype=np.float32)    # [384, 768]
        woTp = np.ascontiguousarray(
            woT.reshape(MC, P, HID).transpose(1, 0, 2).reshape(P, MC * HID))
        in_maps.append({
            "xTp": _pack_x(hs[b].T).astype(np.float16),
            "wp": _pack_w(Wq, Wk, Wv, cols).astype(np.float16),
            "woT": woTp.astype(np.float16),
            "csn": csn,
            "trineg": trineg,
            "eye": eye,
        })
    return in_maps


def run(hidden_states, Wq, Wk, Wv, Wo, trace=False, **trace_kw):
    nc = _get_program()
    in_maps = _make_in_maps(hidden_states, Wq, Wk, Wv, Wo)
    res = run_bass_kernel_spmd(nc, in_maps, core_ids=list(range(N_CORES)),
                               trace=trace, **trace_kw)
    B = 4
    out = np.empty((B, S, HID), dtype=np.float32)
    for b in range(B):
        out[b] = res.results[2 * b]["out"] + res.results[2 * b + 1]["out"]
    return out, res


def kernel(hidden_states, Wq, Wk, Wv, Wo):
    out, _ = run(hidden_states, Wq, Wk, Wv, Wo,
                 trace=bool(int(os.environ.get("KERNEL_TRACE", "0"))))
    return out


# revision 26
# speedup vs baseline: 1.0356x; 1.0356x over previous
"""Trainium2 Bass kernel for HNet attention (B=4, S=2048, H=768, 12 heads, RoPE, causal).

Sharding: 8 cores = 4 batches x 2 head-groups (6 heads each).
Wq/Wk/Wv split column-wise (head axis), Wo row-wise; host sums the two
partial o_proj outputs per batch (the "all-reduce" done at gather time).

Per-core dataflow (v9 — fp16, 256-wide q strips, kc-pair merged exp):
  xT [768,2048] fp16 (host-packed) --PE--> Q,K,V natural [2048,384]
  RoPE on Q,K natural (DVE muls, Pool add), PE-transpose -> qT,kT fp16
  scoresT[k,q] = kT.T @ qT per (pair m, par); causal mask folded into PE
    as an accumulate-matmul of a -30000 triangle; two kc blocks share one
    [128,1024] psum tile so one ScalarE exp covers 4 regions
  ex = exp(0.125*scores - 4.5) fp16 (bias cancels in softmax, keeps the
    self-attention diagonal e^{~14} inside fp16 range)
  PV natural per q-tile: po[q, 65] += ex.T @ [V_h | 1] (col 64 = sums),
    one psum accumulation group per head at a time
  normalize: DVE copy psum->sbuf + reciprocal, per-head scale on Pool
  deferred fill: PE-transpose -> aoT, o_proj fin = aoT.T @ woT, store.
"""

import os
import sys

import numpy as np

sys.path.insert(0, "/opt/trn_rl_repo")

from collections import deque
from contextlib import ExitStack

import concourse.bacc as bacc
import concourse.tile as tile
from concourse import mybir
from concourse.bass_utils import run_bass_kernel_spmd

S = 2048
HID = 768
NH = 6            # heads per core
D = 64
F = NH * D        # 384 per-core feature slice
P = 128
SC = S // P       # 16 s-tiles
FC = HID // P     # 6 contraction chunks
MC = F // P       # 3 head-pair chunks
QW = 256          # q strip width
NQ = S // QW      # 8 strips
QT = QW // P      # 2 q-tiles per strip
N_CORES = 8
ROPE_THETA = 10000.0
NEG = -30000.0
EBIAS = -4.5      # exp bias: cancels in softmax, keeps ex within fp16

F32 = mybir.dt.float32
F16 = mybir.dt.float16
AF = mybir.ActivationFunctionType

XW = 4 * FC * QW * 2   # packed xT width 12288 (4 col-chunks x 6 f x 512)
XCH = FC * 512         # 3072 per chunk
WW = FC * 3 * F        # packed wqkv width 6912
CW = SC * 2 * F        # packed cos|sin width 12288


def build_program():
    nc = bacc.Bacc("TRN2", target_bir_lowering=False, debug=False,
                   num_devices=N_CORES)

    xTp_d = nc.dram_tensor("xTp", [P, XW], F16, kind="ExternalInput").ap()
    wp_d = nc.dram_tensor("wp", [P, WW], F16, kind="ExternalInput").ap()
    woT_d = nc.dram_tensor("woT", [P, MC * HID], F16, kind="ExternalInput").ap()
    csn_d = nc.dram_tensor("csn", [P, CW], F16, kind="ExternalInput").ap()
    tri_d = nc.dram_tensor("trineg", [P, P], F16, kind="ExternalInput").ap()
    eye_d = nc.dram_tensor("eye", [P, P], F16, kind="ExternalInput").ap()
    out_d = nc.dram_tensor("out", [S, HID], F32, kind="ExternalOutput").ap()

    with tile.TileContext(nc) as tc, ExitStack() as ctx:
        const_pool = ctx.enter_context(tc.tile_pool(name="const", bufs=1))
        eye_sb = const_pool.tile([P, P], F16, tag="eye")
        nc.sync.dma_start(eye_sb[:], eye_d[:])
        tri_sb = const_pool.tile([P, P], F16, tag="tri")
        nc.sync.dma_start(tri_sb[:], tri_d[:])
        eb_sb = const_pool.tile([P, 1], F32, tag="ebias")
        nc.gpsimd.memset(eb_sb[:], EBIAS)

        # ---- persistent SBUF; DMA order feeds the prologue first ----
        xw_pool = ctx.enter_context(tc.tile_pool(name="xw", bufs=1))
        wp = xw_pool.tile([P, WW], F16, tag="wp")
        xTp = xw_pool.tile([P, XW], F16, tag="xTp")
        csn = xw_pool.tile([P, CW], F16, tag="csn")
        woT = xw_pool.tile([P, MC * HID], F16, tag="woT")
        for f in range(FC):
            c0 = f * 3 * F
            nc.sync.dma_start(wp[:, c0:c0 + 3 * F], wp_d[:, c0:c0 + 3 * F])
            if f == 2:
                nc.sync.dma_start(xTp[:, 0:XCH // 2], xTp_d[:, 0:XCH // 2])
                nc.sync.dma_start(csn[:, 0:2 * F], csn_d[:, 0:2 * F])
        nc.sync.dma_start(xTp[:, XCH // 2:XCH], xTp_d[:, XCH // 2:XCH])
        nc.sync.dma_start(csn[:, 2 * F:CW // 4], csn_d[:, 2 * F:CW // 4])
        nc.sync.dma_start(woT[:], woT_d[:])

        def wslice(kind, f):  # 0=q 1=k 2=v
            c0 = f * 3 * F + kind * F
            return wp[:, c0:c0 + F]

        def xslice(f, s):
            c0 = (s // 4) * XCH + f * 512 + (s % 4) * P
            return xTp[:, c0:c0 + P]

        kT_pool = ctx.enter_context(tc.tile_pool(name="kTp", bufs=1))
        kTs = kT_pool.tile([P, MC * S], F16, tag="kTs")
        v_pool = ctx.enter_context(tc.tile_pool(name="vp", bufs=1))
        vo = [v_pool.tile([P, NH * 65], F16, tag=f"v{s}", name=f"v{s}")
              for s in range(SC)]
        for s in range(SC):
            v3 = vo[s].rearrange("p (h e) -> p h e", h=NH)
            nc.gpsimd.memset(v3[:, :, 64], 1.0)

        with tc.tile_pool(name="rp", bufs=3) as rp_pool, \
             tc.tile_pool(name="qr", bufs=4) as qr_pool, \
             tc.tile_pool(name="qTs", bufs=4) as qTs_pool, \
             tc.tile_pool(name="ao", bufs=4) as ao_pool, \
             tc.tile_pool(name="ex", bufs=34) as ex_pool, \
             tc.tile_pool(name="an", bufs=14) as an_pool, \
             tc.tile_pool(name="iv", bufs=3) as iv_pool, \
             tc.tile_pool(name="ob", bufs=2) as ob_pool, \
             tc.tile_pool(name="mx", bufs=2, space="PSUM") as mx, \
             tc.tile_pool(name="sc", bufs=2, space="PSUM") as scp, \
             tc.tile_pool(name="po", bufs=2, space="PSUM") as pop:

            qTs = {}   # strip qc -> [P, MC*QW] tile
            aoT = {}   # strip qc -> [P, MC*QW] tile
            ans = {}   # (qc, qt) -> normalized ao_nat tile
            sps = {}   # (pair_index, m) -> scores psum pair tile
            exs = {}   # (pair_index, m) -> ex tile [P, 1024]
            pvb = {}   # live projB state per s

            def rope(pp, s):
                """psum QKV chunk [P, F] -> rotated fp16 sbuf tile."""
                cs = csn[:, s * 2 * F:s * 2 * F + F]
                sn = csn[:, s * 2 * F + F:s * 2 * F + 2 * F]
                p3 = pp.rearrange("p (h d) -> p h d", h=NH)
                s3 = sn.rearrange("p (h d) -> p h d", h=NH)
                t1 = rp_pool.tile([P, F], F32, tag="t1", name="t1")
                nc.vector.tensor_mul(t1[:], pp[:], cs[:])
                t2 = rp_pool.tile([P, F], F32, tag="t2", name="t2")
                t23 = t2.rearrange("p (h d) -> p h d", h=NH)
                nc.vector.tensor_mul(t23[:, :, 0:32], p3[:, :, 32:64],
                                     s3[:, :, 0:32])
                nc.vector.tensor_mul(t23[:, :, 32:64], p3[:, :, 0:32],
                                     s3[:, :, 32:64])
                qr = qr_pool.tile([P, F], F16, tag="qr", name="qr")
                nc.gpsimd.tensor_add(qr[:], t1[:], t2[:])
                return qr

            def emit_projA(s):
                qc = s // QT
                if s % QT == 0:
                    qTs[qc] = qTs_pool.tile([P, MC * QW], F16, tag="qTs",
                                            name="qTs")
                pq = mx.tile([P, F], F32, tag="mx", name="pq")
                for f in range(FC):
                    nc.tensor.matmul(pq[:], xslice(f, s), wslice(0, f),
                                     start=(f == 0), stop=(f == FC - 1))
                pk = mx.tile([P, F], F32, tag="mx", name="pk")
                for f in range(FC):
                    nc.tensor.matmul(pk[:], xslice(f, s), wslice(1, f),
                                     start=(f == 0), stop=(f == FC - 1))
                qr = rope(pq, s)
                return (s, pk, qr)

            def emit_projB(state):
                s, pk, qr = state
                qc, scol = s // QT, (s % QT) * P
                pv_ = mx.tile([P, F], F32, tag="mx", name="pv")
                for f in range(FC):
                    nc.tensor.matmul(pv_[:], xslice(f, s), wslice(2, f),
                                     start=(f == 0), stop=(f == FC - 1))
                kr = rope(pk, s)
                tq = mx.tile([P, F], F16, tag="mx", name="tpq")
                for m in range(MC):
                    nc.tensor.transpose(tq[:, m * P:(m + 1) * P],
                                        qr[:, m * P:(m + 1) * P], eye_sb[:])
                qd = qTs[qc].rearrange("p (m c) -> p m c", m=MC)
                nc.vector.tensor_copy(qd[:, :, scol:scol + P],
                                      tq.rearrange("p (m c) -> p m c", m=MC))
                tk = mx.tile([P, F], F16, tag="mx", name="tpk")
                for m in range(MC):
                    nc.tensor.transpose(tk[:, m * P:(m + 1) * P],
                                        kr[:, m * P:(m + 1) * P], eye_sb[:])
                kd = kTs.rearrange("p (m c) -> p m c", m=MC)
                nc.vector.tensor_copy(kd[:, :, s * P:(s + 1) * P],
                                      tk.rearrange("p (m c) -> p m c", m=MC))
                v3 = vo[s].rearrange("p (h e) -> p h e", h=NH)
                p3 = pv_.rearrange("p (h d) -> p h d", h=NH)
                nc.vector.tensor_copy(v3[:, :, 0:64], p3[:])

            def emit_scores(qc, kc, m):
                """scores for block kc into the kc-pair psum tile; emit the
                exp(s) when the pair completes (odd kc)."""
                q0, k0 = qc * QW, kc * P
                off = max(0, k0 - q0)
                kcp = kc // 2
                odd = kc & 1
                if not odd:
                    sps[(kcp, m)] = scp.tile([P, 4 * QW], F32, tag="sc",
                                             name="sp")
                sp = sps[(kcp, m)]
                b0 = odd * 2 * QW
                for par in range(2):
                    b = b0 + par * QW
                    d0 = 64 * par
                    lhsT = kTs[d0:d0 + 64, m * S + k0:m * S + k0 + P]
                    rhs = qTs[qc]
                    r0 = m * QW
                    if k0 >= q0:
                        nc.tensor.matmul(sp[:, b + off:b + off + P],
                                         lhsT,
                                         rhs[d0:d0 + 64, r0 + off:r0 + off + P],
                                         start=True, stop=False)
                        nc.tensor.matmul(sp[:, b + off:b + off + P],
                                         eye_sb[:], tri_sb[:],
                                         start=False, stop=True)
                        if off + P < QW:
                            nc.tensor.matmul(sp[:, b + off + P:b + QW],
                                             lhsT,
                                             rhs[d0:d0 + 64, r0 + off + P:
                                                 r0 + QW],
                                             start=True, stop=True)
                    else:
                        nc.tensor.matmul(sp[:, b:b + QW],
                                         lhsT, rhs[d0:d0 + 64, r0:r0 + QW],
                                         start=True, stop=True)
                if odd:
                    ex = ex_pool.tile([P, 4 * QW], F16, tag="ex", name="ex")
                    last_pair = (kc == 2 * qc + 1)
                    if last_pair:
                        # ragged diag pair: separate exp per kc block
                        for o, ofe in ((0, 0), (1, P)):
                            sp3 = sp.rearrange("p (r c) -> p r c", r=4)
                            ex3 = ex.rearrange("p (r c) -> p r c", r=4)
                            nc.scalar.activation(
                                ex3[:, 2 * o:2 * o + 2, ofe:QW],
                                sp3[:, 2 * o:2 * o + 2, ofe:QW],
                                AF.Exp, scale=0.125, bias=eb_sb[:])
                    else:
                        sp3 = sp.rearrange("p (r c) -> p r c", r=4)
                        ex3 = ex.rearrange("p (r c) -> p r c", r=4)
                        nc.scalar.activation(ex3[:], sp3[:],
                                             AF.Exp, scale=0.125, bias=eb_sb[:])
                    exs[(kcp, m)] = ex
                    del sps[(kcp, m)]

            def emit_pv(qc, qt):
                """one psum accumulation group per head at a time."""
                t = QT * qc + qt
                po = pop.tile([P, NH * 65], F32, tag="po", name="po")
                for m in range(MC):
                    for par in range(2):
                        h = 2 * m + par
                        for kc in range(t + 1):
                            ex = exs[(kc // 2, m)]
                            c = (kc & 1) * 2 * QW + par * QW + qt * P
                            nc.tensor.matmul(po[:, h * 65:h * 65 + 65],
                                             ex[:, c:c + P],
                                             vo[kc][:, h * 65:h * 65 + 65],
                                             start=(kc == 0), stop=(kc == t))
                return po

            def emit_norm(qc, qt, po):
                """copy po to SBUF, reciprocal, per-head scale on Pool."""
                pz = iv_pool.tile([P, NH * 65], F32, tag="pz", name="pz")
                nc.vector.tensor_copy(pz[:], po[:])
                pz3 = pz.rearrange("p (h e) -> p h e", h=NH)
                inv = iv_pool.tile([P, NH], F32, tag="inv", name="inv")
                with nc.allow_low_precision(reason="softmax sums"):
                    nc.vector.reciprocal(inv[:], pz3[:, :, 64])
                an = an_pool.tile([P, F], F16, tag="an", name="an")
                for h in range(NH):
                    nc.gpsimd.tensor_scalar_mul(an[:, h * D:(h + 1) * D],
                                                pz3[:, h, 0:D],
                                                inv[:, h:h + 1])
                ans[(qc, qt)] = an

            def emit_oproj(qc, qt):
                """deferred PE fill: transpose ao_nat -> aoT, fin, store."""
                if qt == 0:
                    aoT[qc] = ao_pool.tile([P, MC * QW], F16, tag="aoT",
                                           name="aoT")
                an = ans.pop((qc, qt))
                ta = mx.tile([P, F], F16, tag="mx", name="tpa")
                for m in range(MC):
                    nc.tensor.transpose(ta[:, m * P:(m + 1) * P],
                                        an[:, m * P:(m + 1) * P], eye_sb[:])
                ad = aoT[qc].rearrange("p (m c) -> p m c", m=MC)
                nc.vector.tensor_copy(ad[:, :, qt * P:(qt + 1) * P],
                                      ta.rearrange("p (m c) -> p m c", m=MC))
                ob = ob_pool.tile([P, HID], F32, tag="ob", name="ob")
                for half in range(2):
                    c0 = half * F
                    fin = scp.tile([P, F], F32, tag="sc", name="fin")
                    for m in range(MC):
                        nc.tensor.matmul(fin[:],
                                         aoT[qc][:, m * QW + qt * P:
                                                 m * QW + (qt + 1) * P],
                                         woT[:, m * HID + c0:m * HID + c0 + F],
                                         start=(m == 0), stop=(m == MC - 1))
                    nc.vector.tensor_copy(ob[:, c0:c0 + F], fin[:])
                s0 = (QT * qc + qt) * P
                nc.sync.dma_start(out_d[s0:s0 + P, :], ob[:])

            # ---- emission schedule ----
            fills = deque()
            for s in range(2, SC):
                fills.append(("A", s))
                fills.append(("B", s))

            cur_qc = [0]

            def pop_fill():
                if not fills:
                    return
                if fills[0][0] == "O" and cur_qc[0] < 5:
                    return
                kind, a = fills.popleft()
                if kind == "A":
                    pvb[a] = emit_projA(a)
                elif kind == "B":
                    emit_projB(pvb.pop(a))
                else:
                    emit_oproj(*a)

            for s in range(2):
                st = emit_projA(s)
                emit_projB(st)
            for c in range(1, 4):
                nc.sync.dma_start(xTp[:, c * XCH:(c + 1) * XCH],
                                  xTp_d[:, c * XCH:(c + 1) * XCH])
                nc.sync.dma_start(csn[:, c * (CW // 4):(c + 1) * (CW // 4)],
                                  csn_d[:, c * (CW // 4):(c + 1) * (CW // 4)])

            for qc in range(NQ):
                cur_qc[0] = qc
                exs.clear()
                last = QT * qc + 1
                # strip qc's scores read qTs[qc] whole: its proj must be done
                while any(k in ("A", "B") and a <= last for k, a in fills):
                    pop_fill()
                for kcp in range(qc + 1):
                    for m in range(MC):
                        if m < 2:
                            pop_fill()
                        emit_scores(qc, 2 * kcp, m)
                        emit_scores(qc, 2 * kcp + 1, m)
                for qt in range(QT):
                    po = emit_pv(qc, qt)
                    emit_norm(qc, qt, po)
                    fills.append(("O", (qc, qt)))
            cur_qc[0] = NQ
            while fills:
                pop_fill()
    nc.compile()
    return nc


def _rope_tables():
    inv_freq = 1.0 / (ROPE_THETA ** (np.arange(0, D, 2, dtype=np.float32) / D))
    t = np.arange(S, dtype=np.float32)
    freqs = np.outer(t, inv_freq)                       # [S, 32]
    emb = np.concatenate([freqs, freqs], axis=-1)       # [S, 64]
    cos = np.cos(emb).astype(np.float32)
    sin = np.sin(emb).astype(np.float32)
    sin_signed = sin.copy()
    sin_signed[:, 0:32] *= -1.0                         # fold rotate_half sign
    cos6 = np.tile(cos, (1, NH))                        # [S, 384]
    sin6 = np.tile(sin_signed, (1, NH))
    both = np.concatenate(
        [cos6.reshape(SC, P, F), sin6.reshape(SC, P, F)], axis=2)
    return np.ascontiguousarray(
        both.transpose(1, 0, 2).reshape(P, CW)).astype(np.float16)


_STATE = {}


def _get_program():
    if "nc" not in _STATE:
        _STATE["nc"] = build_program()
    return _STATE["nc"]


def _pack_x(xT):
    """[768, 2048] -> [128, 12288] with cols (chunk, f, 512)."""
    v = xT.reshape(FC, P, 4, 512)                # f, p, c, col
    return np.ascontiguousarray(
        v.transpose(1, 2, 0, 3).reshape(P, XW))  # p, (c f col)


def _pack_w(Wq, Wk, Wv, cols):
    ws = [np.asarray(W[cols, :].T, dtype=np.float32).reshape(FC, P, F)
          for W in (Wq, Wk, Wv)]
    stk = np.stack(ws, axis=2)                   # f, p, kind, 384
    return np.ascontiguousarray(stk.transpose(1, 0, 2, 3).reshape(P, WW))


def _make_in_maps(hidden_states, Wq, Wk, Wv, Wo):
    hs = np.asarray(hidden_states, dtype=np.float32)
    Wq = np.asarray(Wq, dtype=np.float32)
    Wk = np.asarray(Wk, dtype=np.float32)
    Wv = np.asarray(Wv, dtype=np.float32)
    Wo = np.asarray(Wo, dtype=np.float32)

    csn = _rope_tables()
    trineg = (NEG * np.tril(np.ones((P, P), dtype=np.float32), -1)
              ).astype(np.float16)
    eye = np.eye(P, dtype=np.float16)

    in_maps = []
    for c in range(N_CORES):
        b, g = c // 2, c % 2
        cols = slice(g * F, (g + 1) * F)
        woT = np.asarray(Wo[:, cols].T, dtype=np.float32)    # [384, 768]
        woTp = np.ascontiguousarray(
            woT.reshape(MC, P, HID).transpose(1, 0, 2).reshape(P, MC * HID))
        in_maps.append({
            "xTp": _pack_x(hs[b].T).astype(np.float16),
            "wp": _pack_w(Wq, Wk, Wv, cols).astype(np.float16),
            "woT": woTp.astype(np.float16),
            "csn": csn,
            "trineg": trineg,
            "eye": eye,
        })
    return in_maps


def run(hidden_states, Wq, Wk, Wv, Wo, trace=False, **trace_kw):
    nc = _get_program()
    in_maps = _make_in_maps(hidden_states, Wq, Wk, Wv, Wo)
    res = run_bass_kernel_spmd(nc, in_maps, core_ids=list(range(N_CORES)),
                               trace=trace, **trace_kw)
    B = 4
    out = np.empty((B, S, HID), dtype=np.float32)
    for b in range(B):
        out[b] = res.results[2 * b]["out"] + res.results[2 * b + 1]["out"]
    return out, res


def kernel(hidden_states, Wq, Wk, Wv, Wo):
    out, _ = run(hidden_states, Wq, Wk, Wv, Wo,
                 trace=bool(int(os.environ.get("KERNEL_TRACE", "0"))))
    return out


# revision 28
# speedup vs baseline: 1.0357x; 1.0001x over previous
"""Trainium2 Bass kernel for HNet attention (B=4, S=2048, H=768, 12 heads, RoPE, causal).

Sharding: 8 cores = 4 batches x 2 head-groups (6 heads each).
Wq/Wk/Wv split column-wise (head axis), Wo row-wise; host sums the two
partial o_proj outputs per batch (the "all-reduce" done at gather time).

Per-core dataflow (v9 — fp16, 256-wide q strips, kc-pair merged exp):
  xT [768,2048] fp16 (host-packed) --PE--> Q,K,V natural [2048,384]
  RoPE on Q,K natural (DVE muls, Pool add), PE-transpose -> qT,kT fp16
  scoresT[k,q] = kT.T @ qT per (pair m, par); causal mask folded into PE
    as an accumulate-matmul of a -30000 triangle; two kc blocks share one
    [128,1024] psum tile so one ScalarE exp covers 4 regions
  ex = exp(0.125*scores - 4.5) fp16 (bias cancels in softmax, keeps the
    self-attention diagonal e^{~14} inside fp16 range)
  PV natural per q-tile: po[q, 65] += ex.T @ [V_h | 1] (col 64 = sums),
    one psum accumulation group per head at a time
  normalize: DVE copy psum->sbuf + reciprocal, per-head scale on Pool
  deferred fill: PE-transpose -> aoT, o_proj fin = aoT.T @ woT, store.
"""

import os
import sys

import numpy as np

sys.path.insert(0, "/opt/trn_rl_repo")

from collections import deque
from contextlib import ExitStack

import concourse.bacc as bacc
import concourse.tile as tile
from concourse import mybir
from concourse.bass_utils import run_bass_kernel_spmd

S = 2048
HID = 768
NH = 6            # heads per core
D = 64
F = NH * D        # 384 per-core feature slice
P = 128
SC = S // P       # 16 s-tiles
FC = HID // P     # 6 contraction chunks
MC = F // P       # 3 head-pair chunks
QW = 256          # q strip width
NQ = S // QW      # 8 strips
QT = QW // P      # 2 q-tiles per strip
N_CORES = 8
ROPE_THETA = 10000.0
NEG = -30000.0
EBIAS = -4.5      # exp bias: cancels in softmax, keeps ex within fp16

F32 = mybir.dt.float32
F16 = mybir.dt.float16
AF = mybir.ActivationFunctionType

XW = 4 * FC * QW * 2   # packed xT width 12288 (4 col-chunks x 6 f x 512)
XCH = FC * 512         # 3072 per chunk
WW = FC * 3 * F        # packed wqkv width 6912
CW = SC * 2 * F        # packed cos|sin width 12288


def build_program():
    nc = bacc.Bacc("TRN2", target_bir_lowering=False, debug=False,
                   num_devices=N_CORES)

    xTp_d = nc.dram_tensor("xTp", [P, XW], F16, kind="ExternalInput").ap()
    wp_d = nc.dram_tensor("wp", [P, WW], F16, kind="ExternalInput").ap()
    woT_d = nc.dram_tensor("woT", [P, MC * HID], F16, kind="ExternalInput").ap()
    csn_d = nc.dram_tensor("csn", [P, CW], F16, kind="ExternalInput").ap()
    tri_d = nc.dram_tensor("trineg", [P, P], F16, kind="ExternalInput").ap()
    eye_d = nc.dram_tensor("eye", [P, P], F16, kind="ExternalInput").ap()
    out_d = nc.dram_tensor("out", [S, HID], F32, kind="ExternalOutput").ap()

    with tile.TileContext(nc) as tc, ExitStack() as ctx:
        const_pool = ctx.enter_context(tc.tile_pool(name="const", bufs=1))
        eye_sb = const_pool.tile([P, P], F16, tag="eye")
        nc.sync.dma_start(eye_sb[:], eye_d[:])
        tri_sb = const_pool.tile([P, P], F16, tag="tri")
        nc.sync.dma_start(tri_sb[:], tri_d[:])
        eb_sb = const_pool.tile([P, 1], F32, tag="ebias")
        nc.gpsimd.memset(eb_sb[:], EBIAS)

        # ---- persistent SBUF; DMA order feeds the prologue first ----
        xw_pool = ctx.enter_context(tc.tile_pool(name="xw", bufs=1))
        wp = xw_pool.tile([P, WW], F16, tag="wp")
        xTp = xw_pool.tile([P, XW], F16, tag="xTp")
        csn = xw_pool.tile([P, CW], F16, tag="csn")
        woT = xw_pool.tile([P, MC * HID], F16, tag="woT")
        for f in range(FC):
            c0 = f * 3 * F
            nc.sync.dma_start(wp[:, c0:c0 + 3 * F], wp_d[:, c0:c0 + 3 * F])
            if f == 2:
                nc.sync.dma_start(xTp[:, 0:XCH // 2], xTp_d[:, 0:XCH // 2])
                nc.sync.dma_start(csn[:, 0:2 * F], csn_d[:, 0:2 * F])
        nc.sync.dma_start(xTp[:, XCH // 2:XCH], xTp_d[:, XCH // 2:XCH])
        nc.sync.dma_start(csn[:, 2 * F:CW // 4], csn_d[:, 2 * F:CW // 4])
        nc.sync.dma_start(woT[:], woT_d[:])

        def wslice(kind, f):  # 0=q 1=k 2=v
            c0 = f * 3 * F + kind * F
            return wp[:, c0:c0 + F]

        def xslice(f, s):
            c0 = (s // 4) * XCH + f * 512 + (s % 4) * P
            return xTp[:, c0:c0 + P]

        kT_pool = ctx.enter_context(tc.tile_pool(name="kTp", bufs=1))
        kTs = kT_pool.tile([P, MC * S], F16, tag="kTs")
        v_pool = ctx.enter_context(tc.tile_pool(name="vp", bufs=1))
        vo = [v_pool.tile([P, NH * 65], F16, tag=f"v{s}", name=f"v{s}")
              for s in range(SC)]
        for s in range(SC):
            v3 = vo[s].rearrange("p (h e) -> p h e", h=NH)
            nc.gpsimd.memset(v3[:, :, 64], 1.0)

        with tc.tile_pool(name="rp", bufs=3) as rp_pool, \
             tc.tile_pool(name="qr", bufs=3) as qr_pool, \
             tc.tile_pool(name="qTs", bufs=4) as qTs_pool, \
             tc.tile_pool(name="ao", bufs=4) as ao_pool, \
             tc.tile_pool(name="ex", bufs=36) as ex_pool, \
             tc.tile_pool(name="an", bufs=10) as an_pool, \
             tc.tile_pool(name="iv", bufs=3) as iv_pool, \
             tc.tile_pool(name="ob", bufs=2) as ob_pool, \
             tc.tile_pool(name="mx", bufs=2, space="PSUM") as mx, \
             tc.tile_pool(name="sc", bufs=2, space="PSUM") as scp, \
             tc.tile_pool(name="po", bufs=2, space="PSUM") as pop:

            qTs = {}   # strip qc -> [P, MC*QW] tile
            aoT = {}   # strip qc -> [P, MC*QW] tile
            ans = {}   # (qc, qt) -> normalized ao_nat tile
            sps = {}   # (pair_index, m) -> scores psum pair tile
            exs = {}   # (pair_index, m) -> ex tile [P, 1024]
            pvb = {}   # live projB state per s

            def rope(pp, s):
                """psum QKV chunk [P, F] -> rotated fp16 sbuf tile."""
                cs = csn[:, s * 2 * F:s * 2 * F + F]
                sn = csn[:, s * 2 * F + F:s * 2 * F + 2 * F]
                p3 = pp.rearrange("p (h d) -> p h d", h=NH)
                s3 = sn.rearrange("p (h d) -> p h d", h=NH)
                t1 = rp_pool.tile([P, F], F32, tag="t1", name="t1")
                nc.vector.tensor_mul(t1[:], pp[:], cs[:])
                t2 = rp_pool.tile([P, F], F32, tag="t2", name="t2")
                t23 = t2.rearrange("p (h d) -> p h d", h=NH)
                nc.vector.tensor_mul(t23[:, :, 0:32], p3[:, :, 32:64],
                                     s3[:, :, 0:32])
                nc.vector.tensor_mul(t23[:, :, 32:64], p3[:, :, 0:32],
                                     s3[:, :, 32:64])
                qr = qr_pool.tile([P, F], F16, tag="qr", name="qr")
                nc.gpsimd.tensor_add(qr[:], t1[:], t2[:])
                return qr

            def emit_projA(s):
                qc = s // QT
                if s % QT == 0:
                    qTs[qc] = qTs_pool.tile([P, MC * QW], F16, tag="qTs",
                                            name="qTs")
                pq = mx.tile([P, F], F32, tag="mx", name="pq")
                for f in range(FC):
                    nc.tensor.matmul(pq[:], xslice(f, s), wslice(0, f),
                                     start=(f == 0), stop=(f == FC - 1))
                pk = mx.tile([P, F], F32, tag="mx", name="pk")
                for f in range(FC):
                    nc.tensor.matmul(pk[:], xslice(f, s), wslice(1, f),
                                     start=(f == 0), stop=(f == FC - 1))
                qr = rope(pq, s)
                return (s, pk, qr)

            def emit_projB(state):
                s, pk, qr = state
                qc, scol = s // QT, (s % QT) * P
                pv_ = mx.tile([P, F], F32, tag="mx", name="pv")
                for f in range(FC):
                    nc.tensor.matmul(pv_[:], xslice(f, s), wslice(2, f),
                                     start=(f == 0), stop=(f == FC - 1))
                kr = rope(pk, s)
                tq = mx.tile([P, F], F16, tag="mx", name="tpq")
                for m in range(MC):
                    nc.tensor.transpose(tq[:, m * P:(m + 1) * P],
                                        qr[:, m * P:(m + 1) * P], eye_sb[:])
                qd = qTs[qc].rearrange("p (m c) -> p m c", m=MC)
                nc.vector.tensor_copy(qd[:, :, scol:scol + P],
                                      tq.rearrange("p (m c) -> p m c", m=MC))
                tk = mx.tile([P, F], F16, tag="mx", name="tpk")
                for m in range(MC):
                    nc.tensor.transpose(tk[:, m * P:(m + 1) * P],
                                        kr[:, m * P:(m + 1) * P], eye_sb[:])
                kd = kTs.rearrange("p (m c) -> p m c", m=MC)
                nc.vector.tensor_copy(kd[:, :, s * P:(s + 1) * P],
                                      tk.rearrange("p (m c) -> p m c", m=MC))
                v3 = vo[s].rearrange("p (h e) -> p h e", h=NH)
                p3 = pv_.rearrange("p (h d) -> p h d", h=NH)
                nc.vector.tensor_copy(v3[:, :, 0:64], p3[:])

            def emit_scores(qc, kc, m):
                """scores for block kc into the kc-pair psum tile; emit the
                exp(s) when the pair completes (odd kc)."""
                q0, k0 = qc * QW, kc * P
                off = max(0, k0 - q0)
                kcp = kc // 2
                odd = kc & 1
                if not odd:
                    sps[(kcp, m)] = scp.tile([P, 4 * QW], F32, tag="sc",
                                             name="sp")
                sp = sps[(kcp, m)]
                b0 = odd * 2 * QW
                for par in range(2):
                    b = b0 + par * QW
                    d0 = 64 * par
                    lhsT = kTs[d0:d0 + 64, m * S + k0:m * S + k0 + P]
                    rhs = qTs[qc]
                    r0 = m * QW
                    if k0 >= q0:
                        nc.tensor.matmul(sp[:, b + off:b + off + P],
                                         lhsT,
                                         rhs[d0:d0 + 64, r0 + off:r0 + off + P],
                                         start=True, stop=False)
                        nc.tensor.matmul(sp[:, b + off:b + off + P],
                                         eye_sb[:], tri_sb[:],
                                         start=False, stop=True)
                        if off + P < QW:
                            nc.tensor.matmul(sp[:, b + off + P:b + QW],
                                             lhsT,
                                             rhs[d0:d0 + 64, r0 + off + P:
                                                 r0 + QW],
                                             start=True, stop=True)
                    else:
                        nc.tensor.matmul(sp[:, b:b + QW],
                                         lhsT, rhs[d0:d0 + 64, r0:r0 + QW],
                                         start=True, stop=True)
                if odd:
                    ex = ex_pool.tile([P, 4 * QW], F16, tag="ex", name="ex")
                    last_pair = (kc == 2 * qc + 1)
                    if last_pair:
                        # ragged diag pair: separate exp per kc block
                        for o, ofe in ((0, 0), (1, P)):
                            sp3 = sp.rearrange("p (r c) -> p r c", r=4)
                            ex3 = ex.rearrange("p (r c) -> p r c", r=4)
                            nc.scalar.activation(
                                ex3[:, 2 * o:2 * o + 2, ofe:QW],
                                sp3[:, 2 * o:2 * o + 2, ofe:QW],
                                AF.Exp, scale=0.125, bias=eb_sb[:])
                    else:
                        sp3 = sp.rearrange("p (r c) -> p r c", r=4)
                        ex3 = ex.rearrange("p (r c) -> p r c", r=4)
                        nc.scalar.activation(ex3[:], sp3[:],
                                             AF.Exp, scale=0.125, bias=eb_sb[:])
                    exs[(qc, kcp, m)] = ex
                    del sps[(kcp, m)]

            def emit_pv(qc, qt):
                """one psum accumulation group per head at a time."""
                t = QT * qc + qt
                po = pop.tile([P, NH * 65], F32, tag="po", name="po")
                for m in range(MC):
                    for par in range(2):
                        h = 2 * m + par
                        for kc in range(t + 1):
                            ex = exs[(qc, kc // 2, m)]
                            c = (kc & 1) * 2 * QW + par * QW + qt * P
                            nc.tensor.matmul(po[:, h * 65:h * 65 + 65],
                                             ex[:, c:c + P],
                                             vo[kc][:, h * 65:h * 65 + 65],
                                             start=(kc == 0), stop=(kc == t))
                return po

            def emit_norm(qc, qt, po):
                """copy po to SBUF, reciprocal, per-head scale on Pool."""
                pz = iv_pool.tile([P, NH * 65], F32, tag="pz", name="pz")
                nc.vector.tensor_copy(pz[:], po[:])
                pz3 = pz.rearrange("p (h e) -> p h e", h=NH)
                inv = iv_pool.tile([P, NH], F32, tag="inv", name="inv")
                with nc.allow_low_precision(reason="softmax sums"):
                    nc.vector.reciprocal(inv[:], pz3[:, :, 64])
                an = an_pool.tile([P, F], F16, tag="an", name="an")
                for h in range(NH):
                    nc.gpsimd.tensor_scalar_mul(an[:, h * D:(h + 1) * D],
                                                pz3[:, h, 0:D],
                                                inv[:, h:h + 1])
                ans[(qc, qt)] = an

            def emit_oproj(qc, qt):
                """deferred PE fill: transpose ao_nat -> aoT, fin, store."""
                if qt == 0:
                    aoT[qc] = ao_pool.tile([P, MC * QW], F16, tag="aoT",
                                           name="aoT")
                an = ans.pop((qc, qt))
                ta = mx.tile([P, F], F16, tag="mx", name="tpa")
                for m in range(MC):
                    nc.tensor.transpose(ta[:, m * P:(m + 1) * P],
                                        an[:, m * P:(m + 1) * P], eye_sb[:])
                ad = aoT[qc].rearrange("p (m c) -> p m c", m=MC)
                nc.vector.tensor_copy(ad[:, :, qt * P:(qt + 1) * P],
                                      ta.rearrange("p (m c) -> p m c", m=MC))
                ob = ob_pool.tile([P, HID], F32, tag="ob", name="ob")
                for half in range(2):
                    c0 = half * F
                    fin = scp.tile([P, F], F32, tag="sc", name="fin")
                    for m in range(MC):
                        nc.tensor.matmul(fin[:],
                                         aoT[qc][:, m * QW + qt * P:
                                                 m * QW + (qt + 1) * P],
                                         woT[:, m * HID + c0:m * HID + c0 + F],
                                         start=(m == 0), stop=(m == MC - 1))
                    nc.vector.tensor_copy(ob[:, c0:c0 + F], fin[:])
                s0 = (QT * qc + qt) * P
                nc.sync.dma_start(out_d[s0:s0 + P, :], ob[:])

            # ---- emission schedule ----
            proj = deque()
            for s in range(2, SC):
                proj.append(("A", s))
                proj.append(("B", s))
            pend = deque()   # deferred PV/norm units from the previous strip
            oq = deque()     # deferred o_proj units
            cur_qc = [0]
            pos = {}         # (qc, qt) -> po tile awaiting norm

            def run_unit(kind, a):
                if kind == "A":
                    pvb[a] = emit_projA(a)
                elif kind == "B":
                    emit_projB(pvb.pop(a))
                elif kind == "P":
                    pos[a] = emit_pv(*a)
                elif kind == "N":
                    emit_norm(*a, pos.pop(a))
                    oq.append(a)
                else:
                    emit_oproj(*a)

            def pop_fill(allow_o=True):
                if pend:
                    run_unit(*pend.popleft())
                elif proj:
                    run_unit(*proj.popleft())
                elif oq and allow_o and cur_qc[0] >= 4:
                    run_unit("O", oq.popleft())

            for s in range(2):
                st = emit_projA(s)
                emit_projB(st)
            for c in range(1, 4):
                nc.sync.dma_start(xTp[:, c * XCH:(c + 1) * XCH],
                                  xTp_d[:, c * XCH:(c + 1) * XCH])
                nc.sync.dma_start(csn[:, c * (CW // 4):(c + 1) * (CW // 4)],
                                  csn_d[:, c * (CW // 4):(c + 1) * (CW // 4)])

            for qc in range(NQ):
                cur_qc[0] = qc
                last = QT * qc + 1
                # strip qc's scores read qTs[qc] whole: its proj must be done
                while pend:
                    run_unit(*pend.popleft())
                while any(a <= last for _, a in proj):
                    run_unit(*proj.popleft())
                for kcp in range(qc + 1):
                    for m in range(MC):
                        if m < 2:
                            pop_fill(allow_o=(m == 0))
                        emit_scores(qc, 2 * kcp, m)
                        emit_scores(qc, 2 * kcp + 1, m)
                for qt in range(QT):
                    if qc < NQ - 1:
                        pend.append(("P", (qc, qt)))
                        pend.append(("N", (qc, qt)))
                    else:
                        run_unit("P", (qc, qt))
                        run_unit("N", (qc, qt))
            cur_qc[0] = NQ
            while pend:
                run_unit(*pend.popleft())
            while oq:
                run_unit("O", oq.popleft())
    nc.compile()
    return nc


def _rope_tables():
    inv_freq = 1.0 / (ROPE_THETA ** (np.arange(0, D, 2, dtype=np.float32) / D))
    t = np.arange(S, dtype=np.float32)
    freqs = np.outer(t, inv_freq)                       # [S, 32]
    emb = np.concatenate([freqs, freqs], axis=-1)       # [S, 64]
    cos = np.cos(emb).astype(np.float32)
    sin = np.sin(emb).astype(np.float32)
    sin_signed = sin.copy()
    sin_signed[:, 0:32] *= -1.0                         # fold rotate_half sign
    cos6 = np.tile(cos, (1, NH))                        # [S, 384]
    sin6 = np.tile(sin_signed, (1, NH))
    both = np.concatenate(
        [cos6.reshape(SC, P, F), sin6.reshape(SC, P, F)], axis=2)
    return np.ascontiguousarray(
        both.transpose(1, 0, 2).reshape(P, CW)).astype(np.float16)


_STATE = {}


def _get_program():
    if "nc" not in _STATE:
        _STATE["nc"] = build_program()
    return _STATE["nc"]


def _pack_x(xT):
    """[768, 2048] -> [128, 12288] with cols (chunk, f, 512)."""
    v = xT.reshape(FC, P, 4, 512)                # f, p, c, col
    return np.ascontiguousarray(
        v.transpose(1, 2, 0, 3).reshape(P, XW))  # p, (c f col)


def _pack_w(Wq, Wk, Wv, cols):
    ws = [np.asarray(W[cols, :].T, dtype=np.float32).reshape(FC, P, F)
          for W in (Wq, Wk, Wv)]
    stk = np.stack(ws, axis=2)                   # f, p, kind, 384
    return np.ascontiguousarray(stk.transpose(1, 0, 2, 3).reshape(P, WW))


def _make_in_maps(hidden_states, Wq, Wk, Wv, Wo):
    hs = np.asarray(hidden_states, dtype=np.float32)
    Wq = np.asarray(Wq, dtype=np.float32)
    Wk = np.asarray(Wk, dtype=np.float32)
    Wv = np.asarray(Wv, dtype=np.float32)
    Wo = np.asarray(Wo, dtype=np.float32)

    csn = _rope_tables()
    trineg = (NEG * np.tril(np.ones((P, P), dtype=np.float32), -1)
              ).astype(np.float16)
    eye = np.eye(P, dtype=np.float16)

    in_maps = []
    for c in range(N_CORES):
        b, g = c // 2, c % 2
        cols = slice(g * F, (g + 1) * F)
        woT = np.asarray(Wo[:, cols].T, dtype=np.float32)    # [384, 768]
        woTp = np.ascontiguousarray(
            woT.reshape(MC, P, HID).transpose(1, 0, 2).reshape(P, MC * HID))
        in_maps.append({
            "xTp": _pack_x(hs[b].T).astype(np.float16),
            "wp": _pack_w(Wq, Wk, Wv, cols).astype(np.float16),
            "woT": woTp.astype(np.float16),
            "csn": csn,
            "trineg": trineg,
            "eye": eye,
        })
    return in_maps


def run(hidden_states, Wq, Wk, Wv, Wo, trace=False, **trace_kw):
    nc = _get_program()
    in_maps = _make_in_maps(hidden_states, Wq, Wk, Wv, Wo)
    res = run_bass_kernel_spmd(nc, in_maps, core_ids=list(range(N_CORES)),
                               trace=trace, **trace_kw)
    B = 4
    out = np.empty((B, S, HID), dtype=np.float32)
    for b in range(B):
        out[b] = res.results[2 * b]["out"] + res.results[2 * b + 1]["out"]
    return out, res


def kernel(hidden_states, Wq, Wk, Wv, Wo):
    out, _ = run(hidden_states, Wq, Wk, Wv, Wo,
                 trace=bool(int(os.environ.get("KERNEL_TRACE", "0"))))
    return out


# revision 29
# speedup vs baseline: 1.1115x; 1.0733x over previous
"""Trainium2 Bass kernel for HNet attention (B=4, S=2048, H=768, 12 heads, RoPE, causal).

Sharding: 8 cores = 4 batches x 2 head-groups (6 heads each).
Wq/Wk/Wv split column-wise (head axis), Wo row-wise; host sums the two
partial o_proj outputs per batch (the "all-reduce" done at gather time).

Per-core dataflow (v9 — fp16, 256-wide q strips, kc-pair merged exp):
  xT [768,2048] fp16 (host-packed) --PE--> Q,K,V natural [2048,384]
  RoPE on Q,K natural (DVE muls, Pool add), PE-transpose -> qT,kT fp16
  scoresT[k,q] = kT.T @ qT per (pair m, par); causal mask folded into PE
    as an accumulate-matmul of a -30000 triangle; two kc blocks share one
    [128,1024] psum tile so one ScalarE exp covers 4 regions
  ex = exp(0.125*scores - 4.5) fp16 (bias cancels in softmax, keeps the
    self-attention diagonal e^{~14} inside fp16 range)
  PV natural per q-tile: po[q, 65] += ex.T @ [V_h | 1] (col 64 = sums),
    one psum accumulation group per head at a time
  normalize: DVE copy psum->sbuf + reciprocal, per-head scale on Pool
  deferred fill: PE-transpose -> aoT, o_proj fin = aoT.T @ woT, store.
"""

import os
import sys

import numpy as np

sys.path.insert(0, "/opt/trn_rl_repo")

from collections import deque
from contextlib import ExitStack

import concourse.bacc as bacc
import concourse.tile as tile
from concourse import mybir
from concourse.bass_utils import run_bass_kernel_spmd

S = 2048
HID = 768
NH = 6            # heads per core
D = 64
F = NH * D        # 384 per-core feature slice
P = 128
SC = S // P       # 16 s-tiles
FC = HID // P     # 6 contraction chunks
MC = F // P       # 3 head-pair chunks
QW = 256          # q strip width
NQ = S // QW      # 8 strips
QT = QW // P      # 2 q-tiles per strip
N_CORES = 8
ROPE_THETA = 10000.0
NEG = -30000.0
EBIAS = -4.5      # exp bias: cancels in softmax, keeps ex within fp16

F32 = mybir.dt.float32
F16 = mybir.dt.float16
AF = mybir.ActivationFunctionType

XW = 4 * FC * QW * 2   # packed xT width 12288 (4 col-chunks x 6 f x 512)
XCH = FC * 512         # 3072 per chunk
WW = FC * 3 * F        # packed wqkv width 6912
CW = SC * 2 * F        # packed cos|sin width 12288


def build_program():
    nc = bacc.Bacc("TRN2", target_bir_lowering=False, debug=False,
                   num_devices=N_CORES)

    xTp_d = nc.dram_tensor("xTp", [P, XW], F16, kind="ExternalInput").ap()
    wp_d = nc.dram_tensor("wp", [P, WW], F16, kind="ExternalInput").ap()
    woT_d = nc.dram_tensor("woT", [P, MC * HID], F16, kind="ExternalInput").ap()
    csn_d = nc.dram_tensor("csn", [P, CW], F16, kind="ExternalInput").ap()
    tri_d = nc.dram_tensor("trineg", [P, P], F16, kind="ExternalInput").ap()
    eye_d = nc.dram_tensor("eye", [P, P], F16, kind="ExternalInput").ap()
    out_d = nc.dram_tensor("out", [S, HID], F32, kind="ExternalOutput").ap()

    with tile.TileContext(nc) as tc, ExitStack() as ctx:
        const_pool = ctx.enter_context(tc.tile_pool(name="const", bufs=1))
        eye_sb = const_pool.tile([P, P], F16, tag="eye")
        nc.sync.dma_start(eye_sb[:], eye_d[:])
        tri_sb = const_pool.tile([P, P], F16, tag="tri")
        nc.sync.dma_start(tri_sb[:], tri_d[:])
        eb_sb = const_pool.tile([P, 1], F32, tag="ebias")
        nc.gpsimd.memset(eb_sb[:], EBIAS)

        # ---- persistent SBUF; DMA order feeds the prologue first ----
        xw_pool = ctx.enter_context(tc.tile_pool(name="xw", bufs=1))
        wp = xw_pool.tile([P, WW], F16, tag="wp")
        xTp = xw_pool.tile([P, XW], F16, tag="xTp")
        csn = xw_pool.tile([P, CW], F16, tag="csn")
        woT = xw_pool.tile([P, MC * HID], F16, tag="woT")
        for f in range(FC):
            c0 = f * 3 * F
            nc.sync.dma_start(wp[:, c0:c0 + 3 * F], wp_d[:, c0:c0 + 3 * F])
            if f == 2:
                nc.sync.dma_start(xTp[:, 0:XCH // 2], xTp_d[:, 0:XCH // 2])
                nc.sync.dma_start(csn[:, 0:2 * F], csn_d[:, 0:2 * F])
        nc.sync.dma_start(xTp[:, XCH // 2:XCH], xTp_d[:, XCH // 2:XCH])
        nc.sync.dma_start(csn[:, 2 * F:CW // 4], csn_d[:, 2 * F:CW // 4])
        nc.sync.dma_start(woT[:], woT_d[:])

        def wslice(kind, f):  # 0=q 1=k 2=v
            c0 = f * 3 * F + kind * F
            return wp[:, c0:c0 + F]

        def xslice(f, s):
            c0 = (s // 4) * XCH + f * 512 + (s % 4) * P
            return xTp[:, c0:c0 + P]

        kT_pool = ctx.enter_context(tc.tile_pool(name="kTp", bufs=1))
        kTs = kT_pool.tile([P, MC * S], F16, tag="kTs")
        v_pool = ctx.enter_context(tc.tile_pool(name="vp", bufs=1))
        vo = [v_pool.tile([P, NH * 65], F16, tag=f"v{s}", name=f"v{s}")
              for s in range(SC)]
        for s in range(SC):
            v3 = vo[s].rearrange("p (h e) -> p h e", h=NH)
            nc.gpsimd.memset(v3[:, :, 64], 1.0)

        with tc.tile_pool(name="rp", bufs=3) as rp_pool, \
             tc.tile_pool(name="qr", bufs=3) as qr_pool, \
             tc.tile_pool(name="qTs", bufs=4) as qTs_pool, \
             tc.tile_pool(name="ao", bufs=4) as ao_pool, \
             tc.tile_pool(name="ex", bufs=34) as ex_pool, \
             tc.tile_pool(name="an", bufs=16) as an_pool, \
             tc.tile_pool(name="iv", bufs=3) as iv_pool, \
             tc.tile_pool(name="ob", bufs=2) as ob_pool, \
             tc.tile_pool(name="mx", bufs=2, space="PSUM") as mx, \
             tc.tile_pool(name="sc", bufs=2, space="PSUM") as scp, \
             tc.tile_pool(name="po", bufs=2, space="PSUM") as pop:

            qTs = {}   # strip qc -> [P, MC*QW] tile
            aoT = {}   # strip qc -> [P, MC*QW] tile
            ans = {}   # (qc, qt) -> normalized ao_nat tile
            sps = {}   # (pair_index, m) -> scores psum pair tile
            exs = {}   # (pair_index, m) -> ex tile [P, 1024]
            pvb = {}   # live projB state per s

            def rope(pp, s):
                """psum QKV chunk [P, F] -> rotated fp16 sbuf tile."""
                cs = csn[:, s * 2 * F:s * 2 * F + F]
                sn = csn[:, s * 2 * F + F:s * 2 * F + 2 * F]
                p3 = pp.rearrange("p (h d) -> p h d", h=NH)
                s3 = sn.rearrange("p (h d) -> p h d", h=NH)
                t1 = rp_pool.tile([P, F], F32, tag="t1", name="t1")
                nc.vector.tensor_mul(t1[:], pp[:], cs[:])
                t2 = rp_pool.tile([P, F], F32, tag="t2", name="t2")
                t23 = t2.rearrange("p (h d) -> p h d", h=NH)
                nc.vector.tensor_mul(t23[:, :, 0:32], p3[:, :, 32:64],
                                     s3[:, :, 0:32])
                nc.vector.tensor_mul(t23[:, :, 32:64], p3[:, :, 0:32],
                                     s3[:, :, 32:64])
                qr = qr_pool.tile([P, F], F16, tag="qr", name="qr")
                nc.gpsimd.tensor_add(qr[:], t1[:], t2[:])
                return qr

            def emit_projA(s):
                qc = s // QT
                if s % QT == 0:
                    qTs[qc] = qTs_pool.tile([P, MC * QW], F16, tag="qTs",
                                            name="qTs")
                pq = mx.tile([P, F], F32, tag="mx", name="pq")
                for f in range(FC):
                    nc.tensor.matmul(pq[:], xslice(f, s), wslice(0, f),
                                     start=(f == 0), stop=(f == FC - 1))
                pk = mx.tile([P, F], F32, tag="mx", name="pk")
                for f in range(FC):
                    nc.tensor.matmul(pk[:], xslice(f, s), wslice(1, f),
                                     start=(f == 0), stop=(f == FC - 1))
                qr = rope(pq, s)
                return (s, pk, qr)

            def emit_projB(state):
                s, pk, qr = state
                qc, scol = s // QT, (s % QT) * P
                pv_ = mx.tile([P, F], F32, tag="mx", name="pv")
                for f in range(FC):
                    nc.tensor.matmul(pv_[:], xslice(f, s), wslice(2, f),
                                     start=(f == 0), stop=(f == FC - 1))
                kr = rope(pk, s)
                tq = mx.tile([P, F], F16, tag="mx", name="tpq")
                for m in range(MC):
                    nc.tensor.transpose(tq[:, m * P:(m + 1) * P],
                                        qr[:, m * P:(m + 1) * P], eye_sb[:])
                qd = qTs[qc].rearrange("p (m c) -> p m c", m=MC)
                nc.vector.tensor_copy(qd[:, :, scol:scol + P],
                                      tq.rearrange("p (m c) -> p m c", m=MC))
                tk = mx.tile([P, F], F16, tag="mx", name="tpk")
                for m in range(MC):
                    nc.tensor.transpose(tk[:, m * P:(m + 1) * P],
                                        kr[:, m * P:(m + 1) * P], eye_sb[:])
                kd = kTs.rearrange("p (m c) -> p m c", m=MC)
                nc.vector.tensor_copy(kd[:, :, s * P:(s + 1) * P],
                                      tk.rearrange("p (m c) -> p m c", m=MC))
                v3 = vo[s].rearrange("p (h e) -> p h e", h=NH)
                p3 = pv_.rearrange("p (h d) -> p h d", h=NH)
                nc.vector.tensor_copy(v3[:, :, 0:64], p3[:])

            def emit_scores(qc, kc, m):
                """scores for block kc into the kc-pair psum tile; emit the
                exp(s) when the pair completes (odd kc)."""
                q0, k0 = qc * QW, kc * P
                off = max(0, k0 - q0)
                kcp = kc // 2
                odd = kc & 1
                if not odd:
                    sps[(kcp, m)] = scp.tile([P, 4 * QW], F32, tag="sc",
                                             name="sp")
                sp = sps[(kcp, m)]
                b0 = odd * 2 * QW
                for par in range(2):
                    b = b0 + par * QW
                    d0 = 64 * par
                    lhsT = kTs[d0:d0 + 64, m * S + k0:m * S + k0 + P]
                    rhs = qTs[qc]
                    r0 = m * QW
                    if k0 >= q0:
                        nc.tensor.matmul(sp[:, b + off:b + off + P],
                                         lhsT,
                                         rhs[d0:d0 + 64, r0 + off:r0 + off + P],
                                         start=True, stop=False)
                        nc.tensor.matmul(sp[:, b + off:b + off + P],
                                         eye_sb[:], tri_sb[:],
                                         start=False, stop=True)
                        if off + P < QW:
                            nc.tensor.matmul(sp[:, b + off + P:b + QW],
                                             lhsT,
                                             rhs[d0:d0 + 64, r0 + off + P:
                                                 r0 + QW],
                                             start=True, stop=True)
                    else:
                        nc.tensor.matmul(sp[:, b:b + QW],
                                         lhsT, rhs[d0:d0 + 64, r0:r0 + QW],
                                         start=True, stop=True)
                if odd:
                    ex = ex_pool.tile([P, 4 * QW], F16, tag="ex", name="ex")
                    last_pair = (kc == 2 * qc + 1)
                    if last_pair:
                        # ragged diag pair: separate exp per kc block
                        for o, ofe in ((0, 0), (1, P)):
                            sp3 = sp.rearrange("p (r c) -> p r c", r=4)
                            ex3 = ex.rearrange("p (r c) -> p r c", r=4)
                            nc.scalar.activation(
                                ex3[:, 2 * o:2 * o + 2, ofe:QW],
                                sp3[:, 2 * o:2 * o + 2, ofe:QW],
                                AF.Exp, scale=0.125, bias=eb_sb[:])
                    else:
                        sp3 = sp.rearrange("p (r c) -> p r c", r=4)
                        ex3 = ex.rearrange("p (r c) -> p r c", r=4)
                        nc.scalar.activation(ex3[:], sp3[:],
                                             AF.Exp, scale=0.125, bias=eb_sb[:])
                    exs[(qc, kcp, m)] = ex
                    del sps[(kcp, m)]

            def emit_pv(qc, qt):
                """one psum accumulation group per head at a time."""
                t = QT * qc + qt
                po = pop.tile([P, NH * 65], F32, tag="po", name="po")
                for m in range(MC):
                    for par in range(2):
                        h = 2 * m + par
                        for kc in range(t + 1):
                            ex = exs[(qc, kc // 2, m)]
                            c = (kc & 1) * 2 * QW + par * QW + qt * P
                            nc.tensor.matmul(po[:, h * 65:h * 65 + 65],
                                             ex[:, c:c + P],
                                             vo[kc][:, h * 65:h * 65 + 65],
                                             start=(kc == 0), stop=(kc == t))
                return po

            def emit_norm(qc, qt, po):
                """copy po to SBUF, reciprocal, per-head scale on Pool."""
                pz = iv_pool.tile([P, NH * 65], F32, tag="pz", name="pz")
                nc.vector.tensor_copy(pz[:], po[:])
                pz3 = pz.rearrange("p (h e) -> p h e", h=NH)
                inv = iv_pool.tile([P, NH], F32, tag="inv", name="inv")
                with nc.allow_low_precision(reason="softmax sums"):
                    nc.vector.reciprocal(inv[:], pz3[:, :, 64])
                an = an_pool.tile([P, F], F16, tag="an", name="an")
                for h in range(NH):
                    nc.gpsimd.tensor_scalar_mul(an[:, h * D:(h + 1) * D],
                                                pz3[:, h, 0:D],
                                                inv[:, h:h + 1])
                ans[(qc, qt)] = an

            def emit_oproj(qc, qt):
                """deferred PE fill: transpose ao_nat -> aoT, fin, store."""
                if qt == 0:
                    aoT[qc] = ao_pool.tile([P, MC * QW], F16, tag="aoT",
                                           name="aoT")
                an = ans.pop((qc, qt))
                ta = mx.tile([P, F], F16, tag="mx", name="tpa")
                for m in range(MC):
                    nc.tensor.transpose(ta[:, m * P:(m + 1) * P],
                                        an[:, m * P:(m + 1) * P], eye_sb[:])
                ad = aoT[qc].rearrange("p (m c) -> p m c", m=MC)
                nc.vector.tensor_copy(ad[:, :, qt * P:(qt + 1) * P],
                                      ta.rearrange("p (m c) -> p m c", m=MC))
                ob = ob_pool.tile([P, HID], F32, tag="ob", name="ob")
                for half in range(2):
                    c0 = half * F
                    fin = scp.tile([P, F], F32, tag="sc", name="fin")
                    for m in range(MC):
                        nc.tensor.matmul(fin[:],
                                         aoT[qc][:, m * QW + qt * P:
                                                 m * QW + (qt + 1) * P],
                                         woT[:, m * HID + c0:m * HID + c0 + F],
                                         start=(m == 0), stop=(m == MC - 1))
                    nc.vector.tensor_copy(ob[:, c0:c0 + F], fin[:])
                s0 = (QT * qc + qt) * P
                nc.sync.dma_start(out_d[s0:s0 + P, :], ob[:])

            # ---- emission schedule ----
            proj = deque()
            for s in range(2, SC):
                proj.append(("A", s))
                proj.append(("B", s))
            pend = deque()   # deferred PV/norm units from the previous strip
            oq = deque()     # deferred o_proj units
            cur_qc = [0]
            pos = {}         # (qc, qt) -> po tile awaiting norm

            def run_unit(kind, a):
                if kind == "A":
                    pvb[a] = emit_projA(a)
                elif kind == "B":
                    emit_projB(pvb.pop(a))
                elif kind == "P":
                    pos[a] = emit_pv(*a)
                elif kind == "N":
                    emit_norm(*a, pos.pop(a))
                    oq.append(a)
                else:
                    emit_oproj(*a)

            UNIT_PE = {"A": 1900, "B": 1300, "N": 50, "O": 1150}

            def unit_cost(kind, a):
                if kind == "P":
                    return 162 * (QT * a[0] + a[1] + 1)
                return UNIT_PE[kind]

            credit = [0.0]

            def pop_fill():
                while credit[0] > 0:
                    if pend:
                        kind, a = pend.popleft()
                    elif proj:
                        kind, a = proj.popleft()
                    elif oq:
                        kind, a = "O", oq.popleft()
                    else:
                        return
                    credit[0] -= unit_cost(kind, a)
                    run_unit(kind, a)

            for s in range(2):
                st = emit_projA(s)
                emit_projB(st)
            for c in range(1, 4):
                nc.sync.dma_start(xTp[:, c * XCH:(c + 1) * XCH],
                                  xTp_d[:, c * XCH:(c + 1) * XCH])
                nc.sync.dma_start(csn[:, c * (CW // 4):(c + 1) * (CW // 4)],
                                  csn_d[:, c * (CW // 4):(c + 1) * (CW // 4)])

            for qc in range(NQ):
                cur_qc[0] = qc
                last = QT * qc + 1
                # strip qc's scores read qTs[qc] whole: its proj must be done
                while pend:
                    kind, a = pend.popleft()
                    credit[0] -= unit_cost(kind, a)
                    run_unit(kind, a)
                while any(a <= last for _, a in proj):
                    kind, a = proj.popleft()
                    credit[0] -= unit_cost(kind, a)
                    run_unit(kind, a)
                credit[0] = max(credit[0], -4000.0)
                for kcp in range(qc + 1):
                    for m in range(MC):
                        credit[0] += 1040 - 430
                        pop_fill()
                        emit_scores(qc, 2 * kcp, m)
                        emit_scores(qc, 2 * kcp + 1, m)
                for qt in range(QT):
                    if qc < NQ - 1:
                        pend.append(("P", (qc, qt)))
                        pend.append(("N", (qc, qt)))
                    else:
                        run_unit("P", (qc, qt))
                        run_unit("N", (qc, qt))
            cur_qc[0] = NQ
            while pend:
                run_unit(*pend.popleft())
            while oq:
                run_unit("O", oq.popleft())
    nc.compile()
    return nc


def _rope_tables():
    inv_freq = 1.0 / (ROPE_THETA ** (np.arange(0, D, 2, dtype=np.float32) / D))
    t = np.arange(S, dtype=np.float32)
    freqs = np.outer(t, inv_freq)                       # [S, 32]
    emb = np.concatenate([freqs, freqs], axis=-1)       # [S, 64]
    cos = np.cos(emb).astype(np.float32)
    sin = np.sin(emb).astype(np.float32)
    sin_signed = sin.copy()
    sin_signed[:, 0:32] *= -1.0                         # fold rotate_half sign
    cos6 = np.tile(cos, (1, NH))                        # [S, 384]
    sin6 = np.tile(sin_signed, (1, NH))
    both = np.concatenate(
        [cos6.reshape(SC, P, F), sin6.reshape(SC, P, F)], axis=2)
    return np.ascontiguousarray(
        both.transpose(1, 0, 2).reshape(P, CW)).astype(np.float16)


_STATE = {}


def _get_program():
    if "nc" not in _STATE:
        _STATE["nc"] = build_program()
    return _STATE["nc"]


def _pack_x(xT):
    """[768, 2048] -> [128, 12288] with cols (chunk, f, 512)."""
    v = xT.reshape(FC, P, 4, 512)                # f, p, c, col
    return np.ascontiguousarray(
        v.transpose(1, 2, 0, 3).reshape(P, XW))  # p, (c f col)


def _pack_w(Wq, Wk, Wv, cols):
    ws = [np.asarray(W[cols, :].T, dtype=np.float32).reshape(FC, P, F)
          for W in (Wq, Wk, Wv)]
    stk = np.stack(ws, axis=2)                   # f, p, kind, 384
    return np.ascontiguousarray(stk.transpose(1, 0, 2, 3).reshape(P, WW))


def _make_in_maps(hidden_states, Wq, Wk, Wv, Wo):
    hs = np.asarray(hidden_states, dtype=np.float32)
    Wq = np.asarray(Wq, dtype=np.float32)
    Wk = np.asarray(Wk, dtype=np.float32)
    Wv = np.asarray(Wv, dtype=np.float32)
    Wo = np.asarray(Wo, dtype=np.float32)

    csn = _rope_tables()
    trineg = (NEG * np.tril(np.ones((P, P), dtype=np.float32), -1)
              ).astype(np.float16)
    eye = np.eye(P, dtype=np.float16)

    in_maps = []
    for c in range(N_CORES):
        b, g = c // 2, c % 2
        cols = slice(g * F, (g + 1) * F)
        woT = np.asarray(Wo[:, cols].T, dtype=np.float32)    # [384, 768]
        woTp = np.ascontiguousarray(
            woT.reshape(MC, P, HID).transpose(1, 0, 2).reshape(P, MC * HID))
        in_maps.append({
            "xTp": _pack_x(hs[b].T).astype(np.float16),
            "wp": _pack_w(Wq, Wk, Wv, cols).astype(np.float16),
            "woT": woTp.astype(np.float16),
            "csn": csn,
            "trineg": trineg,
            "eye": eye,
        })
    return in_maps


def run(hidden_states, Wq, Wk, Wv, Wo, trace=False, **trace_kw):
    nc = _get_program()
    in_maps = _make_in_maps(hidden_states, Wq, Wk, Wv, Wo)
    res = run_bass_kernel_spmd(nc, in_maps, core_ids=list(range(N_CORES)),
                               trace=trace, **trace_kw)
    B = 4
    out = np.empty((B, S, HID), dtype=np.float32)
    for b in range(B):
        out[b] = res.results[2 * b]["out"] + res.results[2 * b + 1]["out"]
    return out, res


def kernel(hidden_states, Wq, Wk, Wv, Wo):
    out, _ = run(hidden_states, Wq, Wk, Wv, Wo,
                 trace=bool(int(os.environ.get("KERNEL_TRACE", "0"))))
    return out
